# revision 1
# baseline (speedup 1.0000x reference)
"""Box2Mask Bass kernel for 8 TRN2 NeuronCores (axon-tunneled).

Per grid cell and (box, view) group: 2D ball query over projected points
(first NSAMPLE in-ball valid points by index), occupancy-weighted mean of
the top-2 feature scores, softmax -> mask pixel.

Device program (SPMD over 8 cores; each core owns 6 of the 48 grid rows):
  - builds per-group point features A_g = [cx, cy, cx^2+cy^2+BIG*inval, 1]
    on device from tiny uploads (~75KB/core: xyz rows, top2-feature deltas,
    box bounds, per-group affine coefficients) instead of shipping a 9.4MB
    replicated coefficient tensor (the axon tunnel is latency-bound),
  - per 128-point block: score matmul vs cell features, prefix-capped
    first-16 selection via a triangular matmul, with the (carry, sum,
    count) state accumulated in a persistent PSUM bank, and
  - a fused finalize (sigmoid gate) producing all 18x288 outputs in one DMA.

The compiled jit(shard_map(bass_exec)) callable is built once per process
and cached (same lowering path run_bass_kernel_spmd uses under axon, made
reusable so per-call jit re-trace/re-compile is not paid), as is the
device-resident zero block for the NEFF's output operand.
"""
import numpy as np
from contextlib import ExitStack

import jax
import concourse.bass as bass
import concourse.tile as tile
from concourse import bacc, mybir

# problem constants (hardcoded per contract)
N = 4096          # points
C = 20            # feature channels
K = 6             # boxes
M = 3             # views
G = K * M         # 18 groups
RES = 48          # H = W
NCORES = 8
SROWS = RES // NCORES          # 6 grid rows per core
SLOC = SROWS * RES             # 288 cells per core
NBLK = N // 128                # 32 point blocks
NSAMPLE = 16
RADIUS2 = 9.0
BIG = 65536.0                  # > RADIUS2; pushes invalid points out of every ball

_f32 = mybir.dt.float32
_bf16 = mybir.dt.bfloat16
_f16 = mybir.dt.float16
_ALU = mybir.AluOpType
_ACT = mybir.ActivationFunctionType

NF = 7  # F rows: 6 view-coords, 1 ones


def _build_nc():
    # Inputs are packed into 3 DRAM tensors (fewer per-call host->device
    # buffers over the latency-bound axon tunnel). They are unpacked into
    # separate SBUF tiles via sliced DMAs; the compute program below only
    # ever touches those tiles.
    #  XB [4, N+SLOC]: cols 0:N = [x;y;z;1] point rows, cols N: = B4 cells
    #  DB [128, NBLK]: top2-feature delta of point (b,p) at [p, b]
    #  SMALL [NF, 90]: cols 0:72 W coeffs, 72:78 projection, 78:84 box lo,
    #                  84:90 box hi
    nc = bacc.Bacc("TRN2", target_bir_lowering=False, debug=False, num_devices=NCORES)
    XB = nc.dram_tensor("XB", [4, N + SLOC], _f32, kind="ExternalInput").ap()
    DB = nc.dram_tensor("DB", [128, NBLK], _bf16, kind="ExternalInput").ap()
    SMALL = nc.dram_tensor("SMALL", [NF, 90], _f32, kind="ExternalInput").ap()
    OUT = nc.dram_tensor("OUT", [G, SLOC], _f16, kind="ExternalOutput").ap()

    with ExitStack() as ctx:
        tc = ctx.enter_context(tile.TileContext(nc))
        consts = ctx.enter_context(tc.tile_pool(name="consts", bufs=1))
        bxpool = ctx.enter_context(tc.tile_pool(name="bxpool", bufs=2))
        stpool = ctx.enter_context(tc.tile_pool(name="stpool", bufs=2))
        agpool = ctx.enter_context(tc.tile_pool(name="agpool", bufs=2))
        wpool = ctx.enter_context(tc.tile_pool(name="wpool", bufs=4))
        spool = ctx.enter_context(tc.tile_pool(name="spool", bufs=4))
        cpool = ctx.enter_context(tc.tile_pool(name="cpool", bufs=4))
        scpool = ctx.enter_context(tc.tile_pool(name="scpool", bufs=2))
        buildps = ctx.enter_context(
            tc.tile_pool(name="bps", bufs=2, space=bass.MemorySpace.PSUM))
        qps = ctx.enter_context(
            tc.tile_pool(name="qps", bufs=1, space=bass.MemorySpace.PSUM))
        pspool = ctx.enter_context(
            tc.tile_pool(name="ps", bufs=3, space=bass.MemorySpace.PSUM))
        statps = ctx.enter_context(
            tc.tile_pool(name="sps", bufs=2, space=bass.MemorySpace.PSUM))

        # ---- load inputs (unpack packed DRAM tensors into separate tiles)
        x4 = consts.tile([4, N], _f32)
        nc.sync.dma_start(x4[:], XB[:, 0:N])
        b4sb = consts.tile([4, SLOC], _f32)
        nc.sync.dma_start(b4sb[:], XB[:, N:N + SLOC])
        db = consts.tile([128, NBLK], _bf16)
        nc.sync.dma_start(db[:], DB)
        wall = consts.tile([NF, 4 * G], _f32)
        nc.sync.dma_start(wall[:], SMALL[:, 0:4 * G])
        proj6 = consts.tile([4, 2 * M], _f32)
        nc.sync.dma_start(proj6[:], SMALL[0:4, 4 * G:4 * G + 2 * M])
        blo = consts.tile([3, K], _f32)
        nc.sync.dma_start(blo[:], SMALL[0:3, 78:84])
        bhi = consts.tile([3, K], _f32)
        nc.sync.dma_start(bhi[:], SMALL[0:3, 84:90])

        # ---- device-generated constants
        # P3[:, b, :] = per-block stationary [128, 3] with cols [0, d, 1]
        p3 = consts.tile([128, NBLK, 3], _bf16)
        nc.vector.memset(p3[:, :, 0], 0.0)
        nc.vector.tensor_copy(p3[:, :, 1], db[:])
        nc.vector.memset(p3[:, :, 2], 1.0)
        # TRI: strict upper ones, diag = -NSAMPLE
        ones128 = consts.tile([128, 128], _bf16)
        nc.gpsimd.memset(ones128[:], 1.0)
        m16 = consts.tile([128, 128], _bf16)
        nc.gpsimd.memset(m16[:], -float(NSAMPLE))
        tri = consts.tile([128, 128], _bf16)
        nc.gpsimd.affine_select(out=tri[:], in_=ones128[:], pattern=[[1, 128]],
                                base=0, channel_multiplier=-1,
                                compare_op=_ALU.is_gt, fill=0.0)
        d16 = consts.tile([128, 128], _bf16)
        nc.gpsimd.affine_select(out=d16[:], in_=m16[:], pattern=[[1, 128]],
                                base=0, channel_multiplier=-1,
                                compare_op=_ALU.is_equal, fill=0.0)
        nc.gpsimd.tensor_tensor(tri[:], tri[:], d16[:], _ALU.add)
        # w3: col0 = 1 (carry += block within count)
        w3 = consts.tile([128, 3], _bf16)
        nc.vector.memset(w3[:, 0:1], 1.0)
        nc.vector.memset(w3[:, 1:3], 0.0)
        car1 = consts.tile([1, 128], _bf16)
        nc.vector.memset(car1[:], 1.0)
        ones31 = consts.tile([3, 1], _f32)
        nc.vector.memset(ones31[:], 1.0)
        ones21 = consts.tile([2, 1], _f32)
        nc.vector.memset(ones21[:], 1.0)
        bigsc = consts.tile([1, 1], _f32)
        nc.vector.memset(bigsc[:], BIG)

        # ---- F tile [NF=7, N]: rows 0-5 view coords, 6 ones.
        # Compute-engine accesses must start at partition 0/32/64/96, so
        # rows are computed in base-0 staging tiles and DMA-placed
        # (DMA has no partition-base restriction).
        f7 = consts.tile([NF, N], _f32)
        for c in range(8):
            sl = slice(512 * c, 512 * (c + 1))
            cm_ps = buildps.tile([2 * M, 512], _f32, tag="b")
            nc.tensor.matmul(cm_ps[:], proj6[:], x4[:, sl], start=True, stop=True)
            cm_st = bxpool.tile([2 * M, 512], _f32, tag="cm")
            nc.scalar.activation(cm_st[:], cm_ps[:], _ACT.Copy)
            nc.sync.dma_start(f7[0:6, sl], cm_st[:])
        nc.sync.dma_start(f7[6:7, :], x4[3:4, :])

        # ---- main loop: per box build inv mask; per group build
        # A_g = W_g.T @ F (rows cx, cy, 0, 1), overwrite row 2 with
        # cx^2 + cy^2 + BIG*inv, then blockwise prefix-capped selection
        sd_t = consts.tile([G, SLOC], _f32)
        cnt_t = consts.tile([G, SLOC], _f32)
        for k in range(K):
            inv = stpool.tile([1, N], _f32, tag="inv")
            for c in range(8):
                sl = slice(512 * c, 512 * (c + 1))
                ge = bxpool.tile([3, 512], _f32, tag="ge")
                nc.vector.tensor_scalar(ge[:], x4[0:3, sl], blo[:, k:k + 1],
                                        None, _ALU.is_ge)
                le = bxpool.tile([3, 512], _f32, tag="le")
                nc.vector.tensor_scalar(le[:], x4[0:3, sl], bhi[:, k:k + 1],
                                        None, _ALU.is_le)
                nc.vector.tensor_tensor(ge[:], ge[:], le[:], _ALU.mult)
                cnt_ps = buildps.tile([1, 512], _f32, tag="b")
                nc.tensor.matmul(cnt_ps[:], ones31[:], ge[:],
                                 start=True, stop=True)
                nc.vector.tensor_scalar(inv[0:1, sl], cnt_ps[:],
                                        2.5, None, _ALU.is_lt)
            for m in range(M):
                g = k * M + m
                a_g = agpool.tile([4, N], _f32, tag="ag")
                qfull = stpool.tile([1, N], _f32, tag="qf")
                for c in range(8):
                    sl = slice(512 * c, 512 * (c + 1))
                    a_ps = buildps.tile([4, 512], _f32, tag="b")
                    nc.tensor.matmul(a_ps[:], wall[:, 4 * g:4 * g + 4],
                                     f7[:, sl], start=True, stop=True)
                    nc.vector.tensor_copy(a_g[:, sl], a_ps[:])
                    sq2 = bxpool.tile([2, 512], _f32, tag="sq2")
                    nc.vector.tensor_tensor(sq2[:], a_g[0:2, sl], a_g[0:2, sl],
                                            _ALU.mult)
                    q_ps = qps.tile([1, 512], _f32, tag="q")
                    nc.tensor.matmul(q_ps[:], ones21[:], sq2[:],
                                     start=True, stop=False)
                    nc.tensor.matmul(q_ps[:], bigsc[:], inv[0:1, sl],
                                     start=False, stop=True)
                    nc.scalar.activation(qfull[0:1, sl], q_ps[:], _ACT.Copy)

                nc.sync.dma_start(a_g[2:3, :], qfull[:])
                state_ps = statps.tile([3, SLOC], _f32)
                carry = None
                for b in range(NBLK):
                    score_ps = pspool.tile([128, SLOC], _f32, tag="ps")
                    nc.tensor.matmul(score_ps[:],
                                     a_g[:, 128 * b:128 * (b + 1)],
                                     b4sb[:], start=True, stop=True)
                    within = wpool.tile([128, SLOC], _bf16)
                    nc.vector.tensor_scalar(within[:], score_ps[:], 0.0, None,
                                            _ALU.is_gt)
                    u_ps = pspool.tile([128, SLOC], _f32, tag="ps")
                    nc.tensor.matmul(u_ps[:], tri[:], within[:],
                                     start=True, stop=(b == 0))
                    if b > 0:
                        nc.tensor.matmul(u_ps[:], car1[:], carry[:],
                                         start=False, stop=True)
                    sel = spool.tile([128, SLOC], _bf16)
                    nc.vector.tensor_scalar(sel[:], u_ps[:], 0.0, None,
                                            _ALU.is_lt)
                    nc.tensor.matmul(state_ps[:], p3[:, b, :], sel[:],
                                     start=(b == 0), stop=False)
                    nc.tensor.matmul(state_ps[:], w3[:], within[:],
                                     start=False, stop=(b == NBLK - 1))
                    if b < NBLK - 1:
                        carry = cpool.tile([1, SLOC], _bf16)
                        nc.scalar.activation(carry[:], state_ps[0:1, :],
                                             _ACT.Copy)
                sc3 = scpool.tile([3, SLOC], _f32)
                nc.vector.tensor_copy(sc3[:], state_ps[:])
                nc.sync.dma_start(sd_t[g:g + 1, :], sc3[1:2, :])
                nc.sync.dma_start(cnt_t[g:g + 1, :], sc3[2:3, :])

        # ---- finalize: out = (cnt>0) * 255 * sigmoid(sd / max(cnt,1))
        cntc = consts.tile([G, SLOC], _f32)
        nc.vector.tensor_scalar(cntc[:], cnt_t[:], 1.0, None, _ALU.max)
        rcp = consts.tile([G, SLOC], _f32)
        nc.vector.reciprocal(rcp[:], cntc[:])
        nfd = consts.tile([G, SLOC], _f32)
        nc.vector.tensor_tensor(nfd[:], sd_t[:], rcp[:], _ALU.mult)
        sig = consts.tile([G, SLOC], _f32)
        nc.scalar.activation(sig[:], nfd[:], _ACT.Sigmoid)
        gate = consts.tile([G, SLOC], _f32)
        nc.vector.tensor_scalar(gate[:], cnt_t[:], 0.5, 255.0,
                                _ALU.is_gt, _ALU.mult)
        orow = consts.tile([G, SLOC], _f16)
        nc.vector.tensor_tensor(orow[:], sig[:], gate[:], _ALU.mult)
        nc.sync.dma_start(OUT, orow[:])
    nc.compile()
    return nc


_nc_cache = None
_exec_cache = None
_zeros_cache = None


def _get_executable():
    """Build the Bass module once and wrap it in a persistently cached
    jit(shard_map(...)) callable (same lowering path run_bass_kernel_spmd
    uses under axon, but reusable across calls so trace/compile is paid
    only once)."""
    global _nc_cache, _exec_cache
    if _exec_cache is not None:
        return _exec_cache
    from concourse.bass2jax import (install_neuronx_cc_hook, _bass_exec_p,
                                    partition_id_tensor)
    from jax.sharding import Mesh, PartitionSpec
    from jax.experimental.shard_map import shard_map

    if _nc_cache is None:
        _nc_cache = _build_nc()
    nc = _nc_cache
    install_neuronx_cc_hook()
    partition_name = nc.partition_id_tensor.name if nc.partition_id_tensor else None
    in_names, out_names, out_avals = [], [], []
    for alloc in nc.m.functions[0].allocations:
        if not isinstance(alloc, mybir.MemoryLocationSet):
            continue
        name = alloc.memorylocations[0].name
        if alloc.kind == "ExternalInput":
            if name != partition_name:
                in_names.append(name)
        elif alloc.kind == "ExternalOutput":
            out_names.append(name)
            out_avals.append(jax.core.ShapedArray(
                tuple(alloc.tensor_shape), mybir.dt.np(alloc.dtype)))
    n_params = len(in_names)
    bind_names = list(in_names) + out_names
    if partition_name is not None:
        bind_names.append(partition_name)

    def _body(*args):
        operands = list(args)
        if partition_name is not None:
            operands.append(partition_id_tensor())
        outs = _bass_exec_p.bind(
            *operands, out_avals=tuple(out_avals), in_names=tuple(bind_names),
            out_names=tuple(out_names), lowering_input_output_aliases=(),
            sim_require_finite=True, sim_require_nnan=True, nc=nc)
        return tuple(outs)

    devices = jax.devices()[:NCORES]
    mesh = Mesh(np.asarray(devices), ("core",))
    nin = n_params + len(out_names)
    sharded = jax.jit(
        shard_map(_body, mesh=mesh, in_specs=(PartitionSpec("core"),) * nin,
                  out_specs=(PartitionSpec("core"),) * len(out_names),
                  check_rep=False),
        keep_unused=True)
    _exec_cache = (sharded, in_names, out_names, out_avals, mesh)
    return _exec_cache


def kernel(xyz, features, boxes, theta, phi, res):
    global _zeros_cache
    xyz = np.ascontiguousarray(np.asarray(xyz, np.float32)[0])       # (N,3)
    features = np.asarray(features, np.float32)[0]                   # (N,C)
    boxes = np.asarray(boxes, np.float32)[0]                         # (K,6)
    theta = np.asarray(theta, np.float64)
    phi = np.asarray(phi, np.float64)
    res = int(res)
    H = W = res

    # ---- host prep (tiny: O(N*(K+M+C)) numpy)
    sint, cost = np.sin(theta), np.cos(theta)
    sinp, cosp = np.sin(phi), np.cos(phi)
    U = np.stack([-sint, cost, np.zeros_like(theta)], -1)            # (M,3)
    V = np.stack([cost * sinp, sint * sinp, cosp], -1)               # (M,3)
    center3 = np.stack([cost * cosp, sint * cosp, sinp], -1)         # (M,3)
    Uf, Vf, c3f = U.astype(np.float32), V.astype(np.float32), center3.astype(np.float32)
    # coords per view (host copy only for per-group min/max)
    xc = xyz[None] - c3f[:, None]                                    # (M,N,3)
    cmx = np.einsum('mnd,md->mn', xc, Uf).astype(np.float32)         # (M,N)
    cmy = np.einsum('mnd,md->mn', xc, Vf).astype(np.float32)
    valid = (np.all(xyz[None] <= boxes[:, None, 3:], -1)
             & np.all(xyz[None] >= boxes[:, None, :3], -1))          # (K,N)
    f2 = np.partition(features, C - 2, axis=-1)[:, C - 2:]
    d = (f2[:, 1] - f2[:, 0]).astype(np.float32)                     # (N,)

    half = 0.8 * H / 2                                               # 19.2
    marg = 0.1 * H                                                   # 4.8
    # F rows: 0-5 cm (view-major x,y pairs), 6 ones
    # A rows per group: [cx, cy, 0, 1] with c = alpha*cm + beta; row 2 is
    # overwritten on device with cx^2 + cy^2 + BIG*inv
    WALLh = np.zeros((NF, 4 * G), np.float64)
    for k in range(K):
        vm = valid[k]
        for m in range(M):
            g = k * M + m
            for ax, cm in ((0, cmx[m]), (1, cmy[m])):
                vc = cm[vm]
                cmin = np.float32(vc.min())
                cmax = np.float32(vc.max())
                ctr = np.float32((cmax + cmin) / 2)
                scale = np.float32(max(np.float32(cmax - cmin), np.float32(1e-5)) / 2)
                alpha = half / np.float64(scale)
                beta = -np.float64(ctr) * alpha + half + marg
                WALLh[2 * m + ax, 4 * g + ax] = alpha
                WALLh[6, 4 * g + ax] = beta
            WALLh[6, 4 * g + 3] = 1.0

    SMALL = np.zeros((NF, 90), np.float64)
    SMALL[:, 0:4 * G] = WALLh
    for m in range(M):
        SMALL[0:3, 4 * G + 2 * m] = U[m]
        SMALL[3, 4 * G + 2 * m] = -np.dot(np.float64(c3f[m]), U[m])
        SMALL[0:3, 4 * G + 2 * m + 1] = V[m]
        SMALL[3, 4 * G + 2 * m + 1] = -np.dot(np.float64(c3f[m]), V[m])
    SMALL[0:3, 78:84] = boxes[:, :3].T
    SMALL[0:3, 84:90] = boxes[:, 3:].T
    SMALL = SMALL.astype(np.float32)

    import ml_dtypes
    XR4 = np.concatenate([xyz.T, np.ones((1, N), np.float32)])       # (4,N)
    DB = np.ascontiguousarray(
        d.reshape(NBLK, 128).T.astype(ml_dtypes.bfloat16))           # (128,NBLK)

    gx, gy = np.meshgrid(np.arange(H), np.arange(W), indexing='ij')
    samples = np.stack([gx, gy], -1).reshape(-1, 2).astype(np.float32)
    xbs = []
    for cidx in range(NCORES):
        s = samples[cidx * SLOC:(cidx + 1) * SLOC]
        b4 = np.stack([
            2.0 * s[:, 0], 2.0 * s[:, 1],
            -np.ones(SLOC, np.float32),
            RADIUS2 - (s[:, 0] ** 2 + s[:, 1] ** 2),
        ]).astype(np.float32)
        xbs.append(np.concatenate([XR4, b4], axis=1))                # (4,N+SLOC)

    sharded, in_names, out_names, out_avals, mesh = _get_executable()
    per_core = {"DB": DB, "SMALL": SMALL}
    concat_in = []
    for name in in_names:
        if name == "XB":
            concat_in.append(np.concatenate(xbs, axis=0))
        else:
            a = per_core[name]
            concat_in.append(np.concatenate([a] * NCORES, axis=0))
    if _zeros_cache is None:
        from jax.sharding import NamedSharding, PartitionSpec
        zs = [np.zeros((NCORES * av.shape[0], *av.shape[1:]), av.dtype)
              for av in out_avals]
        _zeros_cache = [jax.device_put(z, NamedSharding(mesh, PartitionSpec("core")))
                        for z in zs]
        for z in _zeros_cache:
            z.block_until_ready()

    outs = sharded(*concat_in, *_zeros_cache)
    arr = np.asarray(outs[0])                                        # (8*G, SLOC)
    full = arr.reshape(NCORES, G, SROWS, W).transpose(1, 0, 2, 3).reshape(G, H, W)
    out = np.broadcast_to(full[:, None, :, :], (G, 3, H, W)).astype(np.float32)
    return np.ascontiguousarray(out)



# revision 5
# speedup vs baseline: 67.1102x; 67.1102x over previous
"""Box2Mask Bass kernel for 8 TRN2 NeuronCores (axon-tunneled).

Per grid cell and (box, view) group: 2D ball query over projected points
(first NSAMPLE in-ball valid points by index), occupancy-weighted mean of
the top-2 feature score deltas, sigmoid -> mask pixel.

v2 device program (SPMD over 8 cores; each core owns 6 of the 48 grid
rows = 288 cells, all 18 groups):
  - the per-group point coefficient matrix A (split-precision bf16 rows
    [cxh cyh cxm cym cxl cyl qh qm ql 1 1 1]) is built EXACTLY on host
    and shipped once per call (~1.8MB bf16, replicated): the score
    matmul runs as ONE bf16 matmul instead of a two-pass fp32 matmul,
    and the old on-device build phase (~0.8ms) disappears entirely.
  - points are laid out 127 per 128-block with partition 0 reserved:
    the running in-ball count (carry) rides in row 0 of the `within`
    mask and the tri matmul both broadcasts it into every prefix AND
    emits the next carry in row 0 of u — no separate carry matmul, no
    w3 count matmul, no scalar-engine copy on the critical path.
  - per (group, block): 3 matmuls total (score, tri, p2-accumulate),
    `within` compare alternates vector/gpsimd, first-16 select is a
    saturated sigmoid on the scalar engine (u is integer-valued, so
    sigmoid(-80*u - 40) is exactly 1 for u<0 and ~4e-18 for u>=0).
"""
import numpy as np
from contextlib import ExitStack

import jax
import concourse.bass as bass
import concourse.tile as tile
from concourse import bacc, mybir

# problem constants (hardcoded per contract)
N = 4096          # points
C = 20            # feature channels
K = 6             # boxes
M = 3             # views
G = K * M         # 18 groups
RES = 48          # H = W
NCORES = 8
SROWS = RES // NCORES          # 6 grid rows per core
SLOC = SROWS * RES             # 288 cells per core
PPB = 127                      # real points per 128-block (partition 0 = carry)
NB = (N + PPB - 1) // PPB      # 33 blocks
N2 = NB * 128                  # 4224 padded point columns
NSAMPLE = 16
RADIUS2 = 9.0
BIG = 65536.0                  # > any valid score; kills invalid/dummy points
CAP = 64.0                     # carry clamp (any value >= NSAMPLE behaves the same)
AR = 12                        # A rows (split-precision contract dim)

_f32 = mybir.dt.float32
_bf16 = mybir.dt.bfloat16
_f16 = mybir.dt.float16
_ALU = mybir.AluOpType
_ACT = mybir.ActivationFunctionType


def _build_nc():
    # DRAM inputs (bf16, packed into 2 tensors):
    #  AB [AR, G*N2]: per-group split-precision A matrices, side by side
    #  XS [128, NB+SLOC]: cols 0:NB = DB (top2-delta per point), rows 0:AR
    #                     of cols NB: = per-cell rhs B4
    nc = bacc.Bacc("TRN2", target_bir_lowering=False, debug=False, num_devices=NCORES)
    AB = nc.dram_tensor("AB", [AR, G * N2], _bf16, kind="ExternalInput").ap()
    XS = nc.dram_tensor("XS", [128, NB + SLOC], _bf16, kind="ExternalInput").ap()
    OUT = nc.dram_tensor("OUT", [G, SLOC], _f16, kind="ExternalOutput").ap()

    with ExitStack() as ctx:
        tc = ctx.enter_context(tile.TileContext(nc))
        consts = ctx.enter_context(tc.tile_pool(name="consts", bufs=1))
        wpool = ctx.enter_context(tc.tile_pool(name="wpool", bufs=3))
        selpool = ctx.enter_context(tc.tile_pool(name="selp", bufs=3))
        scpool = ctx.enter_context(tc.tile_pool(name="scp", bufs=2))
        spsum = ctx.enter_context(
            tc.tile_pool(name="sps", bufs=3, space=bass.MemorySpace.PSUM))
        upsum = ctx.enter_context(
            tc.tile_pool(name="ups", bufs=3, space=bass.MemorySpace.PSUM))
        stps = ctx.enter_context(
            tc.tile_pool(name="stp", bufs=2, space=bass.MemorySpace.PSUM))

        # ---- load inputs
        absb = consts.tile([AR, G * N2], _bf16)
        nc.sync.dma_start(absb[:], AB)
        xssb = consts.tile([128, NB + SLOC], _bf16)
        nc.sync.dma_start(xssb[:], XS)
        b4 = xssb[0:AR, NB:NB + SLOC]

        # ---- device constants
        # p2[:, b, :] = [d, 1] per point of block b (row 0: d = 0 from host)
        p2 = consts.tile([128, NB, 2], _bf16)
        nc.vector.tensor_copy(p2[:, :, 0], xssb[:, 0:NB])
        nc.vector.memset(p2[:, :, 1], 1.0)
        # tri[q, p]: row 0 = 1 (carry broadcast), col 0 = 1 (next carry =
        # carry + block count), strict upper ones (prefix), diag(p>=1) = -16
        ones128 = consts.tile([128, 128], _bf16)
        nc.gpsimd.memset(ones128[:], 1.0)
        m16 = consts.tile([128, 128], _bf16)
        nc.gpsimd.memset(m16[:], -float(NSAMPLE))
        tri = consts.tile([128, 128], _bf16)
        nc.gpsimd.affine_select(out=tri[:], in_=ones128[:], pattern=[[1, 128]],
                                base=0, channel_multiplier=-1,
                                compare_op=_ALU.is_gt, fill=0.0)
        d16 = consts.tile([128, 128], _bf16)
        nc.gpsimd.affine_select(out=d16[:], in_=m16[:], pattern=[[1, 128]],
                                base=0, channel_multiplier=-1,
                                compare_op=_ALU.is_equal, fill=0.0)
        nc.gpsimd.tensor_tensor(tri[:], tri[:], d16[:], _ALU.add)
        nc.vector.memset(tri[0:1, :], 1.0)
        nc.vector.memset(tri[:, 0:1], 1.0)

        sd_t = consts.tile([G, SLOC], _f32)
        cnt_t = consts.tile([G, SLOC], _f32)
        selbias = consts.tile([128, 1], _f32)
        nc.vector.memset(selbias[:], -40.0)

        # ---- main loop: per group, software-pipelined over 33 blocks
        for g in range(G):
            cb = g * N2
            state_ps = stps.tile([2, SLOC], _f32)
            score_t, u_t, sel_t = {}, {}, {}

            def emit_score(b, cb=cb, score_t=score_t):
                t = spsum.tile([128, SLOC], _f32)
                nc.tensor.matmul(t[:], absb[:, cb + 128 * b: cb + 128 * (b + 1)],
                                 b4, start=True, stop=True)
                score_t[b] = t

            emit_score(0)
            emit_score(1)
            for i in range(NB):
                w = wpool.tile([128, SLOC], _bf16)
                nc.vector.tensor_scalar(w[:], score_t[i][:], 0.0, None, _ALU.is_gt)
                del score_t[i]
                if i > 0:
                    nc.vector.tensor_scalar(w[0:1, :], u_t[i - 1][0:1, :],
                                            CAP, None, _ALU.min)
                if i + 2 < NB:
                    emit_score(i + 2)
                if i > 0:
                    nc.tensor.matmul(state_ps[:], p2[:, i - 1, :],
                                     sel_t.pop(i - 1)[:],
                                     start=(i == 1), stop=False)
                u = upsum.tile([128, SLOC], _f32)
                nc.tensor.matmul(u[:], tri[:], w[:], start=True, stop=True)
                u_t[i] = u
                s = selpool.tile([128, SLOC], _bf16)
                nc.scalar.activation(s[:], u[:], _ACT.Sigmoid,
                                     bias=selbias[:], scale=-80.0)
                sel_t[i] = s
            nc.tensor.matmul(state_ps[:], p2[:, NB - 1, :], sel_t.pop(NB - 1)[:],
                             start=False, stop=True)
            sc = scpool.tile([2, SLOC], _f32)
            nc.vector.tensor_copy(sc[:], state_ps[:])
            nc.sync.dma_start(sd_t[g:g + 1, :], sc[0:1, :])
            nc.sync.dma_start(cnt_t[g:g + 1, :], sc[1:2, :])

        # ---- finalize: out = (cnt>0) * 255 * sigmoid(sd / max(cnt,1))
        cntc = consts.tile([G, SLOC], _f32)
        nc.vector.tensor_scalar(cntc[:], cnt_t[:], 1.0, None, _ALU.max)
        rcp = consts.tile([G, SLOC], _f32)
        nc.vector.reciprocal(rcp[:], cntc[:])
        nfd = consts.tile([G, SLOC], _f32)
        nc.vector.tensor_tensor(nfd[:], sd_t[:], rcp[:], _ALU.mult)
        sig = consts.tile([G, SLOC], _f32)
        nc.scalar.activation(sig[:], nfd[:], _ACT.Sigmoid)
        gate = consts.tile([G, SLOC], _f32)
        nc.vector.tensor_scalar(gate[:], cnt_t[:], 0.5, 255.0,
                                _ALU.is_gt, _ALU.mult)
        orow = consts.tile([G, SLOC], _f16)
        nc.vector.tensor_tensor(orow[:], sig[:], gate[:], _ALU.mult)
        nc.sync.dma_start(OUT, orow[:])
    nc.compile()
    return nc


_nc_cache = None
_exec_cache = None
_zeros_cache = None


def _get_nc():
    global _nc_cache
    if _nc_cache is None:
        _nc_cache = _build_nc()
    return _nc_cache


def _split3(x32):
    """Exact 3-way bf16 split of an f32 array: h + m + bf16(l) ~ x32."""
    import ml_dtypes
    bf = ml_dtypes.bfloat16
    h = x32.astype(bf).astype(np.float32)
    r = x32 - h
    m = r.astype(bf).astype(np.float32)
    l = r - m
    return h, m, l


def _host_prep(xyz, features, boxes, theta, phi, res):
    """Build the per-core input maps {AB, XS} from full inputs."""
    import ml_dtypes
    bf = ml_dtypes.bfloat16
    xyz = np.ascontiguousarray(np.asarray(xyz, np.float32)[0])       # (N,3)
    features = np.asarray(features, np.float32)[0]                   # (N,C)
    boxes = np.asarray(boxes, np.float32)[0]                         # (K,6)
    theta = np.asarray(theta, np.float64)
    phi = np.asarray(phi, np.float64)
    H = W = int(res)

    sint, cost = np.sin(theta), np.cos(theta)
    sinp, cosp = np.sin(phi), np.cos(phi)
    U = np.stack([-sint, cost, np.zeros_like(theta)], -1)            # (M,3)
    V = np.stack([cost * sinp, sint * sinp, cosp], -1)               # (M,3)
    center3 = np.stack([cost * cosp, sint * cosp, sinp], -1)         # (M,3)
    Uf, Vf = U.astype(np.float32), V.astype(np.float32)
    c3f = center3.astype(np.float32)
    xc = xyz[None] - c3f[:, None]                                    # (M,N,3)
    cmx = np.einsum('mnd,md->mn', xc, Uf).astype(np.float32)         # (M,N)
    cmy = np.einsum('mnd,md->mn', xc, Vf).astype(np.float32)
    valid = (np.all(xyz[None] <= boxes[:, None, 3:], -1)
             & np.all(xyz[None] >= boxes[:, None, :3], -1))          # (K,N)
    f2 = np.partition(features, C - 2, axis=-1)[:, C - 2:]
    d = (f2[:, 1] - f2[:, 0]).astype(np.float32)                     # (N,)

    half = 0.8 * H / 2
    marg = 0.1 * H

    # per-group scaled coords (f64 affine of the f32 cm, like the device
    # fp32 build chain but with host headroom)
    CX = np.empty((G, N), np.float64)
    CY = np.empty((G, N), np.float64)
    for k in range(K):
        vm = valid[k]
        for m in range(M):
            g = k * M + m
            for ax, cm in ((0, cmx[m]), (1, cmy[m])):
                vc = cm[vm]
                cmin = np.float32(vc.min())
                cmax = np.float32(vc.max())
                ctr = np.float32((cmax + cmin) / 2)
                scale = np.float32(max(np.float32(cmax - cmin),
                                       np.float32(1e-5)) / 2)
                alpha = half / np.float64(scale)
                beta = -np.float64(ctr) * alpha + half + marg
                cc = alpha * cm.astype(np.float64) + beta
                (CX if ax == 0 else CY)[g] = cc

    CXf = CX.astype(np.float32)
    CYf = CY.astype(np.float32)
    Q2 = (CX * CX + CY * CY).astype(np.float32)                      # (G,N)

    cxh, cxm, cxl = _split3(CXf)
    cyh, cym, cyl = _split3(CYf)
    qh, qm, ql = _split3(Q2)

    vG = np.repeat(valid, M, axis=0)                                 # (G,N)
    rows = [
        np.where(vG, cxh, 0.0), np.where(vG, cyh, 0.0),
        np.where(vG, cxm, 0.0), np.where(vG, cym, 0.0),
        np.where(vG, cxl, 0.0), np.where(vG, cyl, 0.0),
        np.where(vG, qh, BIG), np.where(vG, qm, 0.0),
        np.where(vG, ql, 0.0),
    ]

    j = np.arange(N)
    cols = 128 * (j // PPB) + 1 + (j % PPB)                          # dummy col 0 per block
    A = np.zeros((G, AR, N2), np.float32)
    A[:, 6, :] = BIG                                                 # dummy/pad: never in ball
    A[:, 9:12, :] = 1.0
    for r, vals in enumerate(rows):
        A[:, r, cols] = vals
    AB_host = np.ascontiguousarray(
        A.transpose(1, 0, 2).reshape(AR, G * N2)).astype(bf)

    DB = np.zeros((128, NB), np.float32)
    DB[1 + (j % PPB), j // PPB] = d
    DB = DB.astype(bf)

    gx, gy = np.meshgrid(np.arange(H), np.arange(W), indexing='ij')
    samples = np.stack([gx, gy], -1).reshape(-1, 2).astype(np.float32)
    in_maps = []
    for cidx in range(NCORES):
        s = samples[cidx * SLOC:(cidx + 1) * SLOC]
        T = (RADIUS2 - (s[:, 0].astype(np.float64) ** 2
                        + s[:, 1].astype(np.float64) ** 2)).astype(np.float32)
        Th, Tm, Tl = _split3(T)
        b4 = np.stack([
            2.0 * s[:, 0], 2.0 * s[:, 1],
            2.0 * s[:, 0], 2.0 * s[:, 1],
            2.0 * s[:, 0], 2.0 * s[:, 1],
            -np.ones(SLOC, np.float32), -np.ones(SLOC, np.float32),
            -np.ones(SLOC, np.float32),
            Th, Tm, Tl,
        ]).astype(np.float32)                                        # (AR, SLOC)
        XSc = np.zeros((128, NB + SLOC), np.float32)
        XSc[:, 0:NB] = DB.astype(np.float32)
        XSc[0:AR, NB:] = b4
        in_maps.append({"AB": AB_host, "XS": XSc.astype(bf)})
    return in_maps


def _get_executable():
    """Build the Bass module once and wrap it in a persistently cached
    jit(shard_map(...)) callable (same lowering path run_bass_kernel_spmd
    uses under axon, but reusable across calls so trace/compile is paid
    only once)."""
    global _exec_cache
    if _exec_cache is not None:
        return _exec_cache
    from concourse.bass2jax import (install_neuronx_cc_hook, _bass_exec_p,
                                    partition_id_tensor)
    from jax.sharding import Mesh, PartitionSpec
    from jax.experimental.shard_map import shard_map

    nc = _get_nc()
    install_neuronx_cc_hook()
    partition_name = nc.partition_id_tensor.name if nc.partition_id_tensor else None
    in_names, out_names, out_avals = [], [], []
    for alloc in nc.m.functions[0].allocations:
        if not isinstance(alloc, mybir.MemoryLocationSet):
            continue
        name = alloc.memorylocations[0].name
        if alloc.kind == "ExternalInput":
            if name != partition_name:
                in_names.append(name)
        elif alloc.kind == "ExternalOutput":
            out_names.append(name)
            out_avals.append(jax.core.ShapedArray(
                tuple(alloc.tensor_shape), mybir.dt.np(alloc.dtype)))
    n_params = len(in_names)
    bind_names = list(in_names) + out_names
    if partition_name is not None:
        bind_names.append(partition_name)

    def _body(*args):
        operands = list(args)
        if partition_name is not None:
            operands.append(partition_id_tensor())
        outs = _bass_exec_p.bind(
            *operands, out_avals=tuple(out_avals), in_names=tuple(bind_names),
            out_names=tuple(out_names), lowering_input_output_aliases=(),
            sim_require_finite=True, sim_require_nnan=True, nc=nc)
        return tuple(outs)

    devices = jax.devices()[:NCORES]
    mesh = Mesh(np.asarray(devices), ("core",))
    nin = n_params + len(out_names)
    sharded = jax.jit(
        shard_map(_body, mesh=mesh, in_specs=(PartitionSpec("core"),) * nin,
                  out_specs=(PartitionSpec("core"),) * len(out_names),
                  check_rep=False),
        keep_unused=True)
    _exec_cache = (sharded, in_names, out_names, out_avals, mesh)
    return _exec_cache


def kernel(xyz, features, boxes, theta, phi, res):
    global _zeros_cache
    res = int(res)
    H = W = res
    in_maps = _host_prep(xyz, features, boxes, theta, phi, res)

    sharded, in_names, out_names, out_avals, mesh = _get_executable()
    concat_in = [np.concatenate([m[name] for m in in_maps], axis=0)
                 for name in in_names]
    if _zeros_cache is None:
        from jax.sharding import NamedSharding, PartitionSpec
        zs = [np.zeros((NCORES * av.shape[0], *av.shape[1:]), av.dtype)
              for av in out_avals]
        _zeros_cache = [jax.device_put(z, NamedSharding(mesh, PartitionSpec("core")))
                        for z in zs]
        for z in _zeros_cache:
            z.block_until_ready()

    outs = sharded(*concat_in, *_zeros_cache)
    arr = np.asarray(outs[0])                                        # (8*G, SLOC)
    full = arr.reshape(NCORES, G, SROWS, W).transpose(1, 0, 2, 3).reshape(G, H, W)
    out = np.broadcast_to(full[:, None, :, :], (G, 3, H, W)).astype(np.float32)
    return np.ascontiguousarray(out)


# revision 6
# speedup vs baseline: 94.4480x; 1.4074x over previous
"""Box2Mask Bass kernel for 8 TRN2 NeuronCores (axon-tunneled).

Per grid cell and (box, view) group: 2D ball query over projected points
(first NSAMPLE in-ball valid points by index), occupancy-weighted mean of
the top-2 feature score deltas, sigmoid -> mask pixel.

v2 device program (SPMD over 8 cores; each core owns 6 of the 48 grid
rows = 288 cells, all 18 groups):
  - the per-group point coefficient matrix A (split-precision bf16 rows
    [cxh cyh cxm cym cxl cyl qh qm ql 1 1 1]) is built EXACTLY on host
    and shipped once per call (~1.8MB bf16, replicated): the score
    matmul runs as ONE bf16 matmul instead of a two-pass fp32 matmul,
    and the old on-device build phase (~0.8ms) disappears entirely.
  - points are laid out 127 per 128-block with partition 0 reserved:
    the running in-ball count (carry) rides in row 0 of the `within`
    mask and the tri matmul both broadcasts it into every prefix AND
    emits the next carry in row 0 of u — no separate carry matmul, no
    w3 count matmul, no scalar-engine copy on the critical path.
  - per (group, block): 3 matmuls total (score, tri, p2-accumulate),
    `within` compare alternates vector/gpsimd, first-16 select is a
    saturated sigmoid on the scalar engine (u is integer-valued, so
    sigmoid(-80*u - 40) is exactly 1 for u<0 and ~4e-18 for u>=0).
"""
import numpy as np
from contextlib import ExitStack

import jax
import concourse.bass as bass
import concourse.tile as tile
from concourse import bacc, mybir

# problem constants (hardcoded per contract)
N = 4096          # points
C = 20            # feature channels
K = 6             # boxes
M = 3             # views
G = K * M         # 18 groups
RES = 48          # H = W
NCORES = 8
SROWS = RES // NCORES          # 6 grid rows per core
SLOC = SROWS * RES             # 288 cells per core
PPB = 127                      # real points per 128-block (partition 0 = carry)
NB = (N + PPB - 1) // PPB      # 33 blocks
N2 = NB * 128                  # 4224 padded point columns
NSAMPLE = 16
RADIUS2 = 9.0
BIG = 65536.0                  # > any valid score; kills invalid/dummy points
CAP = 64.0                     # carry clamp (any value >= NSAMPLE behaves the same)
AR = 12                        # A rows (split-precision contract dim)

_f32 = mybir.dt.float32
_bf16 = mybir.dt.bfloat16
_f16 = mybir.dt.float16
_ALU = mybir.AluOpType
_ACT = mybir.ActivationFunctionType


def _build_nc():
    # DRAM inputs (bf16, packed into 2 tensors):
    #  AB [AR, G*N2]: per-group split-precision A matrices, side by side
    #  XS [128, NB+SLOC]: cols 0:NB = DB (top2-delta per point), rows 0:AR
    #                     of cols NB: = per-cell rhs B4
    nc = bacc.Bacc("TRN2", target_bir_lowering=False, debug=False, num_devices=NCORES)
    AB = nc.dram_tensor("AB", [AR, G * N2], _bf16, kind="ExternalInput").ap()
    XS = nc.dram_tensor("XS", [128, NB + SLOC], _bf16, kind="ExternalInput").ap()
    OUT = nc.dram_tensor("OUT", [G, SLOC], _f16, kind="ExternalOutput").ap()

    with ExitStack() as ctx:
        tc = ctx.enter_context(tile.TileContext(nc))
        consts = ctx.enter_context(tc.tile_pool(name="consts", bufs=1))
        wpool = ctx.enter_context(tc.tile_pool(name="wpool", bufs=3))
        selpool = ctx.enter_context(tc.tile_pool(name="selp", bufs=3))
        scpool = ctx.enter_context(tc.tile_pool(name="scp", bufs=2))
        spsum = ctx.enter_context(
            tc.tile_pool(name="sps", bufs=3, space=bass.MemorySpace.PSUM))
        upsum = ctx.enter_context(
            tc.tile_pool(name="ups", bufs=3, space=bass.MemorySpace.PSUM))
        stps = ctx.enter_context(
            tc.tile_pool(name="stp", bufs=2, space=bass.MemorySpace.PSUM))

        # ---- load inputs
        absb = consts.tile([AR, G * N2], _bf16)
        nc.sync.dma_start(absb[:], AB)
        xssb = consts.tile([128, NB + SLOC], _bf16)
        nc.sync.dma_start(xssb[:], XS)
        b4 = xssb[0:AR, NB:NB + SLOC]

        # ---- device constants
        # p2[:, b, :] = [d, 1] per point of block b (row 0: d = 0 from host)
        p2 = consts.tile([128, NB, 2], _bf16)
        nc.vector.tensor_copy(p2[:, :, 0], xssb[:, 0:NB])
        nc.vector.memset(p2[:, :, 1], 1.0)
        # tri[q, p]: row 0 = 1 (carry broadcast), col 0 = 1 (next carry =
        # carry + block count), strict upper ones (prefix), diag(p>=1) = -16
        ones128 = consts.tile([128, 128], _bf16)
        nc.gpsimd.memset(ones128[:], 1.0)
        m16 = consts.tile([128, 128], _bf16)
        nc.gpsimd.memset(m16[:], -float(NSAMPLE))
        tri = consts.tile([128, 128], _bf16)
        nc.gpsimd.affine_select(out=tri[:], in_=ones128[:], pattern=[[1, 128]],
                                base=0, channel_multiplier=-1,
                                compare_op=_ALU.is_gt, fill=0.0)
        d16 = consts.tile([128, 128], _bf16)
        nc.gpsimd.affine_select(out=d16[:], in_=m16[:], pattern=[[1, 128]],
                                base=0, channel_multiplier=-1,
                                compare_op=_ALU.is_equal, fill=0.0)
        nc.gpsimd.tensor_tensor(tri[:], tri[:], d16[:], _ALU.add)
        nc.vector.memset(tri[0:1, :], 1.0)
        nc.vector.memset(tri[:, 0:1], 1.0)

        sd_t = consts.tile([G, SLOC], _f32)
        cnt_t = consts.tile([G, SLOC], _f32)
        selbias = consts.tile([128, 1], _f32)
        nc.vector.memset(selbias[:], -40.0)

        # ---- main loop: per group, software-pipelined over 33 blocks
        for g in range(G):
            cb = g * N2
            state_ps = stps.tile([2, SLOC], _f32)
            score_t, u_t, sel_t = {}, {}, {}

            def emit_score(b, cb=cb, score_t=score_t):
                t = spsum.tile([128, SLOC], _f32)
                nc.tensor.matmul(t[:], absb[:, cb + 128 * b: cb + 128 * (b + 1)],
                                 b4, start=True, stop=True)
                score_t[b] = t

            emit_score(0)
            emit_score(1)
            w_t = {}
            for i in range(NB):
                w = wpool.tile([128, SLOC], _bf16)
                nc.vector.tensor_scalar(w[:], score_t[i][:], 0.0, None, _ALU.is_gt)
                del score_t[i]
                if i > 0 and i % 2 == 0:
                    # pair-level carry: u[0] of the previous (odd) block is
                    # the cumulative in-ball count (bf16 rounding above 256
                    # is harmless - only exactness below NSAMPLE matters)
                    nc.vector.tensor_copy(w[0:1, :], u_t[i - 1][0:1, :])
                w_t[i] = w
                if i + 2 < NB:
                    emit_score(i + 2)
                if i > 0:
                    nc.tensor.matmul(state_ps[:], p2[:, i - 1, :],
                                     sel_t.pop(i - 1)[:],
                                     start=(i == 1), stop=False)
                u = upsum.tile([128, SLOC], _f32)
                if i % 2 == 1:
                    # odd block of a pair: carry + even-block total are
                    # injected via a rank-1 ones matmul over w_{i-1}
                    # (w_i row 0 stays 0, so tri contributes no carry)
                    nc.tensor.matmul(u[:], tri[:], w[:], start=True, stop=False)
                    nc.tensor.matmul(u[:], ones128[:], w_t[i - 1][:],
                                     start=False, stop=True)
                    del w_t[i - 1]
                else:
                    nc.tensor.matmul(u[:], tri[:], w[:], start=True, stop=True)
                u_t[i] = u
                s = selpool.tile([128, SLOC], _bf16)
                nc.scalar.activation(s[:], u[:], _ACT.Sigmoid,
                                     bias=selbias[:], scale=-80.0)
                sel_t[i] = s
            nc.tensor.matmul(state_ps[:], p2[:, NB - 1, :], sel_t.pop(NB - 1)[:],
                             start=False, stop=True)
            sc = scpool.tile([2, SLOC], _f32)
            nc.scalar.activation(sc[:], state_ps[:], _ACT.Copy)
            nc.sync.dma_start(sd_t[g:g + 1, :], sc[0:1, :])
            nc.sync.dma_start(cnt_t[g:g + 1, :], sc[1:2, :])

        # ---- finalize: out = (cnt>0) * 255 * sigmoid(sd / max(cnt,1))
        cntc = consts.tile([G, SLOC], _f32)
        nc.vector.tensor_scalar(cntc[:], cnt_t[:], 1.0, None, _ALU.max)
        rcp = consts.tile([G, SLOC], _f32)
        nc.vector.reciprocal(rcp[:], cntc[:])
        nfd = consts.tile([G, SLOC], _f32)
        nc.vector.tensor_tensor(nfd[:], sd_t[:], rcp[:], _ALU.mult)
        sig = consts.tile([G, SLOC], _f32)
        nc.scalar.activation(sig[:], nfd[:], _ACT.Sigmoid)
        gate = consts.tile([G, SLOC], _f32)
        nc.vector.tensor_scalar(gate[:], cnt_t[:], 0.5, 255.0,
                                _ALU.is_gt, _ALU.mult)
        orow = consts.tile([G, SLOC], _f16)
        nc.vector.tensor_tensor(orow[:], sig[:], gate[:], _ALU.mult)
        nc.sync.dma_start(OUT, orow[:])
    nc.compile()
    return nc


_nc_cache = None
_exec_cache = None
_zeros_cache = None


def _get_nc():
    global _nc_cache
    if _nc_cache is None:
        _nc_cache = _build_nc()
    return _nc_cache


def _split3(x32):
    """Exact 3-way bf16 split of an f32 array: h + m + bf16(l) ~ x32."""
    import ml_dtypes
    bf = ml_dtypes.bfloat16
    h = x32.astype(bf).astype(np.float32)
    r = x32 - h
    m = r.astype(bf).astype(np.float32)
    l = r - m
    return h, m, l


def _host_prep(xyz, features, boxes, theta, phi, res):
    """Build the per-core input maps {AB, XS} from full inputs."""
    import ml_dtypes
    bf = ml_dtypes.bfloat16
    xyz = np.ascontiguousarray(np.asarray(xyz, np.float32)[0])       # (N,3)
    features = np.asarray(features, np.float32)[0]                   # (N,C)
    boxes = np.asarray(boxes, np.float32)[0]                         # (K,6)
    theta = np.asarray(theta, np.float64)
    phi = np.asarray(phi, np.float64)
    H = W = int(res)

    sint, cost = np.sin(theta), np.cos(theta)
    sinp, cosp = np.sin(phi), np.cos(phi)
    U = np.stack([-sint, cost, np.zeros_like(theta)], -1)            # (M,3)
    V = np.stack([cost * sinp, sint * sinp, cosp], -1)               # (M,3)
    center3 = np.stack([cost * cosp, sint * cosp, sinp], -1)         # (M,3)
    Uf, Vf = U.astype(np.float32), V.astype(np.float32)
    c3f = center3.astype(np.float32)
    xc = xyz[None] - c3f[:, None]                                    # (M,N,3)
    cmx = np.einsum('mnd,md->mn', xc, Uf).astype(np.float32)         # (M,N)
    cmy = np.einsum('mnd,md->mn', xc, Vf).astype(np.float32)
    valid = (np.all(xyz[None] <= boxes[:, None, 3:], -1)
             & np.all(xyz[None] >= boxes[:, None, :3], -1))          # (K,N)
    f2 = np.partition(features, C - 2, axis=-1)[:, C - 2:]
    d = (f2[:, 1] - f2[:, 0]).astype(np.float32)                     # (N,)

    half = 0.8 * H / 2
    marg = 0.1 * H

    # per-group scaled coords (f64 affine of the f32 cm, like the device
    # fp32 build chain but with host headroom)
    CX = np.empty((G, N), np.float64)
    CY = np.empty((G, N), np.float64)
    for k in range(K):
        vm = valid[k]
        for m in range(M):
            g = k * M + m
            for ax, cm in ((0, cmx[m]), (1, cmy[m])):
                vc = cm[vm]
                cmin = np.float32(vc.min())
                cmax = np.float32(vc.max())
                ctr = np.float32((cmax + cmin) / 2)
                scale = np.float32(max(np.float32(cmax - cmin),
                                       np.float32(1e-5)) / 2)
                alpha = half / np.float64(scale)
                beta = -np.float64(ctr) * alpha + half + marg
                cc = alpha * cm.astype(np.float64) + beta
                (CX if ax == 0 else CY)[g] = cc

    CXf = CX.astype(np.float32)
    CYf = CY.astype(np.float32)
    Q2 = (CX * CX + CY * CY).astype(np.float32)                      # (G,N)

    cxh, cxm, cxl = _split3(CXf)
    cyh, cym, cyl = _split3(CYf)
    qh, qm, ql = _split3(Q2)

    vG = np.repeat(valid, M, axis=0)                                 # (G,N)
    rows = [
        np.where(vG, cxh, 0.0), np.where(vG, cyh, 0.0),
        np.where(vG, cxm, 0.0), np.where(vG, cym, 0.0),
        np.where(vG, cxl, 0.0), np.where(vG, cyl, 0.0),
        np.where(vG, qh, BIG), np.where(vG, qm, 0.0),
        np.where(vG, ql, 0.0),
    ]

    j = np.arange(N)
    cols = 128 * (j // PPB) + 1 + (j % PPB)                          # dummy col 0 per block
    A = np.zeros((G, AR, N2), np.float32)
    A[:, 6, :] = BIG                                                 # dummy/pad: never in ball
    A[:, 9:12, :] = 1.0
    for r, vals in enumerate(rows):
        A[:, r, cols] = vals
    AB_host = np.ascontiguousarray(
        A.transpose(1, 0, 2).reshape(AR, G * N2)).astype(bf)

    DB = np.zeros((128, NB), np.float32)
    DB[1 + (j % PPB), j // PPB] = d
    DB = DB.astype(bf)

    gx, gy = np.meshgrid(np.arange(H), np.arange(W), indexing='ij')
    samples = np.stack([gx, gy], -1).reshape(-1, 2).astype(np.float32)
    in_maps = []
    for cidx in range(NCORES):
        s = samples[cidx * SLOC:(cidx + 1) * SLOC]
        T = (RADIUS2 - (s[:, 0].astype(np.float64) ** 2
                        + s[:, 1].astype(np.float64) ** 2)).astype(np.float32)
        Th, Tm, Tl = _split3(T)
        b4 = np.stack([
            2.0 * s[:, 0], 2.0 * s[:, 1],
            2.0 * s[:, 0], 2.0 * s[:, 1],
            2.0 * s[:, 0], 2.0 * s[:, 1],
            -np.ones(SLOC, np.float32), -np.ones(SLOC, np.float32),
            -np.ones(SLOC, np.float32),
            Th, Tm, Tl,
        ]).astype(np.float32)                                        # (AR, SLOC)
        XSc = np.zeros((128, NB + SLOC), np.float32)
        XSc[:, 0:NB] = DB.astype(np.float32)
        XSc[0:AR, NB:] = b4
        in_maps.append({"AB": AB_host, "XS": XSc.astype(bf)})
    return in_maps


def _get_executable():
    """Build the Bass module once and wrap it in a persistently cached
    jit(shard_map(...)) callable (same lowering path run_bass_kernel_spmd
    uses under axon, but reusable across calls so trace/compile is paid
    only once)."""
    global _exec_cache
    if _exec_cache is not None:
        return _exec_cache
    from concourse.bass2jax import (install_neuronx_cc_hook, _bass_exec_p,
                                    partition_id_tensor)
    from jax.sharding import Mesh, PartitionSpec
    from jax.experimental.shard_map import shard_map

    nc = _get_nc()
    install_neuronx_cc_hook()
    partition_name = nc.partition_id_tensor.name if nc.partition_id_tensor else None
    in_names, out_names, out_avals = [], [], []
    for alloc in nc.m.functions[0].allocations:
        if not isinstance(alloc, mybir.MemoryLocationSet):
            continue
        name = alloc.memorylocations[0].name
        if alloc.kind == "ExternalInput":
            if name != partition_name:
                in_names.append(name)
        elif alloc.kind == "ExternalOutput":
            out_names.append(name)
            out_avals.append(jax.core.ShapedArray(
                tuple(alloc.tensor_shape), mybir.dt.np(alloc.dtype)))
    n_params = len(in_names)
    bind_names = list(in_names) + out_names
    if partition_name is not None:
        bind_names.append(partition_name)

    def _body(*args):
        operands = list(args)
        if partition_name is not None:
            operands.append(partition_id_tensor())
        outs = _bass_exec_p.bind(
            *operands, out_avals=tuple(out_avals), in_names=tuple(bind_names),
            out_names=tuple(out_names), lowering_input_output_aliases=(),
            sim_require_finite=True, sim_require_nnan=True, nc=nc)
        return tuple(outs)

    devices = jax.devices()[:NCORES]
    mesh = Mesh(np.asarray(devices), ("core",))
    nin = n_params + len(out_names)
    sharded = jax.jit(
        shard_map(_body, mesh=mesh, in_specs=(PartitionSpec("core"),) * nin,
                  out_specs=(PartitionSpec("core"),) * len(out_names),
                  check_rep=False),
        keep_unused=True)
    _exec_cache = (sharded, in_names, out_names, out_avals, mesh)
    return _exec_cache


def kernel(xyz, features, boxes, theta, phi, res):
    global _zeros_cache
    res = int(res)
    H = W = res
    in_maps = _host_prep(xyz, features, boxes, theta, phi, res)

    sharded, in_names, out_names, out_avals, mesh = _get_executable()
    concat_in = [np.concatenate([m[name] for m in in_maps], axis=0)
                 for name in in_names]
    if _zeros_cache is None:
        from jax.sharding import NamedSharding, PartitionSpec
        zs = [np.zeros((NCORES * av.shape[0], *av.shape[1:]), av.dtype)
              for av in out_avals]
        _zeros_cache = [jax.device_put(z, NamedSharding(mesh, PartitionSpec("core")))
                        for z in zs]
        for z in _zeros_cache:
            z.block_until_ready()

    outs = sharded(*concat_in, *_zeros_cache)
    arr = np.asarray(outs[0])                                        # (8*G, SLOC)
    full = arr.reshape(NCORES, G, SROWS, W).transpose(1, 0, 2, 3).reshape(G, H, W)
    out = np.broadcast_to(full[:, None, :, :], (G, 3, H, W)).astype(np.float32)
    return np.ascontiguousarray(out)
